# revision 7
# baseline (speedup 1.0000x reference)
"""Trainium2 kernel for quantized GEMV: out = dequant(x) @ dequant(y).

Reference computation (K=4096, N=32768, int8 inputs, f32 output):
    xf = (x - X_ZP) * X_SCALE          # [K]
    yf = (y - Y_ZP) * Y_SCALE          # [K, N]
    out = xf @ yf                      # [N]

Math used on device (exact affine rewrite):
    out[n] = A * sum_k (x[k]-X_ZP) * y[k,n]  +  D
    A = X_SCALE*Y_SCALE,  D = -A * Y_ZP * sum_k (x[k]-X_ZP)

Sharding: y column-sharded across 8 cores ([4096, 4096] per core), x
replicated. Each core computes its 4096-wide output slice; no collectives.

Per-core dataflow (engine specialization, explicit semaphores):
  sync   : HBM->SBUF DMAs of y in 2 MiB chunks (4 k-chunks each), x DMA,
           partition->free gather of x row sums, final output DMA.
  vector : casts even k-chunks int8->bf16; computes x' = x - X_ZP (bf16)
           with row-sum accumulation; computes the scalar bias D.
  scalar : casts odd k-chunks; fused epilogue out = A*psum + D from PSUM.
  tensor : 256 matmuls [128,1]x[128,512] accumulating 8 PSUM banks.

The builder supports `reps`: the y pipeline is repeated in-NEFF with
cumulative semaphore thresholds, for steady-state wall-clock timing.
"""

import sys

for _p in ("/opt/trn_rl_repo", "/root/.axon_site/_ro/trn_rl_repo"):
    if _p not in sys.path:
        sys.path.append(_p)

import numpy as np

import concourse.bass as bass
import concourse.mybir as mybir
from concourse.bass_utils import run_bass_kernel_spmd

X_SCALE, X_ZP = 0.0215, -25
Y_SCALE, Y_ZP = 0.0176, 18
K, N = 4096, 32768
NCORES = 8
NC = N // NCORES            # 4096 columns per core
KC = K // 128               # 32 k-chunks of 128
NJ = NC // 512              # 8 n-chunks of 512 per core
CPD = 4                     # k-chunks per y DMA (2 MiB)
NDMA = KC // CPD            # 8 y DMAs per rep
A_CONST = X_SCALE * Y_SCALE

_cached = {}


def _build_program(reps=1):
    dt = mybir.dt
    nc = bass.Bass("TRN2", target_bir_lowering=False, debug=False,
                   num_devices=NCORES)

    x_ext = nc.declare_dram_parameter("x", [128, KC], dt.int8, isOutput=False)
    y_ext = nc.declare_dram_parameter("y", [K, NC], dt.int8, isOutput=False)
    out_ext = nc.declare_dram_parameter("out", [1, NC], dt.float32,
                                        isOutput=True)

    xs8 = nc.alloc_sbuf_tensor("xs8", [128, KC], dt.int8)
    xw = nc.alloc_sbuf_tensor("xw", [128, KC], dt.bfloat16)
    xsum_p = nc.alloc_sbuf_tensor("xsum_p", [128, 1], dt.float32)
    xsum_t = nc.alloc_sbuf_tensor("xsum_t", [1, 128], dt.float32)
    sig_tmp = nc.alloc_sbuf_tensor("sig_tmp", [1, 128], dt.float32)
    biasv = nc.alloc_sbuf_tensor("biasv", [1, 1], dt.float32)
    ob = nc.alloc_sbuf_tensor("ob", [1, NC], dt.float32)

    # int8 staging: 2 buffers of 4 k-chunks; bf16: 4 buffers of 1 k-chunk
    y8 = [nc.alloc_sbuf_tensor(f"y8_{b}", [128, CPD, NC], dt.int8)
          for b in range(2)]
    yb = [nc.alloc_sbuf_tensor(f"yb_{b}", [128, NC], dt.bfloat16)
          for b in range(4)]
    ps = [nc.alloc_psum_tensor(f"ps_{j}", [1, 512], dt.float32)
          for j in range(NJ)]

    with (
        nc.Block() as block,
        nc.semaphore("s_inx") as s_inx,
        nc.semaphore("s_iny0") as s_iny0,
        nc.semaphore("s_iny1") as s_iny1,
        nc.semaphore("s_cd") as s_cd,
        nc.semaphore("s_ca") as s_ca,
        nc.semaphore("s_pe") as s_pe,
        nc.semaphore("s_xw") as s_xw,
        nc.semaphore("s_sdma") as s_sdma,
        nc.semaphore("s_bias") as s_bias,
        nc.semaphore("s_ep") as s_ep,
        nc.semaphore("s_out") as s_out,
    ):
        @block.sync
        def _(eng: bass.BassEngine):
            eng.dma_start(out=xs8[:], in_=x_ext[:]).then_inc(s_inx, 16)
            # gather per-partition x' sums into one partition (early, so
            # DVE computes the bias before its cast stream begins)
            eng.wait_ge(s_xw, 1)
            eng.dma_start(out=xsum_t[:], in_=xsum_p[:]).then_inc(s_sdma, 16)
            for r in range(reps):
                for c in range(NDMA):
                    g = r * NDMA + c
                    if g >= 2:
                        # staging buf g%2 was consumed by casts of DMA g-2
                        eng.wait_ge(s_cd, 2 * (g - 2) + 2)
                        eng.wait_ge(s_ca, 2 * (g - 2) + 2)
                    src = y_ext[c * 512:(c + 1) * 512, :].rearrange(
                        "(a p) n -> p a n", p=128)
                    eng.dma_start(out=y8[g % 2][:], in_=src).then_inc(
                        s_iny0 if g % 2 == 0 else s_iny1, 16)
                eng.wait_ge(s_ep, NJ * (r + 1))
                eng.dma_start(out=out_ext[:], in_=ob[:]).then_inc(s_out, 16)
            eng.wait_ge(s_out, 16 * reps)

        @block.vector
        def _(eng: bass.BassEngine):
            eng.wait_ge(s_inx, 16)
            # x' = x - X_ZP (exact in bf16), with per-partition row sums
            eng.tensor_scalar(
                xw[:], xs8[:], float(-X_ZP), None, mybir.AluOpType.add,
                mybir.AluOpType.add, accum_out=xsum_p[:],
            ).then_inc(s_xw)
            # scalar bias D = reduce_add(xsum_t * (-A*Y_ZP))
            eng.wait_ge(s_sdma, 16)
            eng.tensor_scalar(
                sig_tmp[:], xsum_t[:], float(-A_CONST * Y_ZP), None,
                mybir.AluOpType.mult, mybir.AluOpType.add,
                accum_out=biasv[:],
            ).then_inc(s_bias)
            for r in range(reps):
                for kc in range(0, KC, 2):  # even k-chunks
                    g = r * NDMA + kc // CPD
                    gk = r * KC + kc
                    eng.wait_ge(s_iny0 if g % 2 == 0 else s_iny1,
                                16 * (g // 2 + 1))
                    if gk >= 4:
                        eng.wait_ge(s_pe, gk - 3)
                    src = y8[g % 2][:, kc % CPD, :]
                    eng.tensor_copy(yb[kc % 4][:], src).then_inc(s_cd)

        @block.scalar
        def _(eng: bass.BassEngine):
            for r in range(reps):
                for kc in range(1, KC, 2):  # odd k-chunks
                    g = r * NDMA + kc // CPD
                    gk = r * KC + kc
                    eng.wait_ge(s_iny0 if g % 2 == 0 else s_iny1,
                                16 * (g // 2 + 1))
                    if gk >= 4:
                        eng.wait_ge(s_pe, gk - 3)
                    src = y8[g % 2][:, kc % CPD, :]
                    eng.copy(yb[kc % 4][:], src).then_inc(s_ca)
                # epilogue: out = A*psum + D
                eng.wait_ge(s_pe, KC * (r + 1))
                if r == 0:
                    eng.wait_ge(s_bias, 1)
                else:
                    # ob may still be read by previous rep's output DMA
                    eng.wait_ge(s_out, 16 * r)
                for j in range(NJ):
                    eng.activation(
                        ob[:, j * 512:(j + 1) * 512], ps[j][:],
                        mybir.ActivationFunctionType.Identity,
                        bias=biasv[:], scale=float(A_CONST),
                    ).then_inc(s_ep)

        @block.tensor
        def _(eng: bass.BassEngine):
            for r in range(reps):
                if r > 0:
                    # PSUM banks still being read by previous epilogue
                    eng.wait_ge(s_ep, NJ * r)
                for kc in range(KC):
                    if kc % 2 == 0:
                        eng.wait_ge(s_cd, r * (KC // 2) + kc // 2 + 1)
                    else:
                        eng.wait_ge(s_ca, r * (KC // 2) + (kc + 1) // 2)
                    for j in range(NJ):
                        mm = eng.matmul(
                            ps[j][:], xw[:, kc:kc + 1],
                            yb[kc % 4][:, j * 512:(j + 1) * 512],
                            start=(kc == 0), stop=(kc == KC - 1),
                        )
                    mm.then_inc(s_pe)

    return nc


def _get_program(reps=1):
    key = ("nc", reps)
    if key not in _cached:
        _cached[key] = _build_program(reps)
    return _cached[key]


def make_in_maps(x, y):
    x = np.asarray(x, dtype=np.int8)
    y = np.asarray(y, dtype=np.int8)
    assert x.shape == (K,) and y.shape == (K, N), (x.shape, y.shape)
    xr = np.ascontiguousarray(x.reshape(KC, 128).T)  # [128, 32]
    return [
        {"x": xr, "y": np.ascontiguousarray(y[:, i * NC:(i + 1) * NC])}
        for i in range(NCORES)
    ]


def run(x, y, reps=1, trace=False):
    in_maps = make_in_maps(x, y)
    nc = _get_program(reps)
    kw = {"trace": True} if trace else {}
    res = run_bass_kernel_spmd(nc, in_maps, core_ids=list(range(NCORES)), **kw)
    out = np.concatenate(
        [np.asarray(res.results[i]["out"]).reshape(NC) for i in range(NCORES)]
    ).astype(np.float32)
    return out, res


def kernel(x, y):
    out, _ = run(x, y)
    return out
